# revision 3
# baseline (speedup 1.0000x reference)
"""Trainium2 Bass kernel for nn_CustomGAT_46033459478728.

3-layer GATv2 (H=8 heads, C=16) + pre-MLP + per-layer mean-pool readout on a
50k-node / 800k-edge random graph, distributed over 8 NeuronCores.

Strategy (dst-sharding):
- Nodes (and their incoming edges) are sharded by dst across 8 cores
  (6250 nodes/core, padded to 6272 = 49 chunks of 128).
- Edges are sorted by dst, grouped into 128-dst-node chunks, padded to a
  uniform T=18 tiles of 128 edges; within a chunk edges are sorted by src so
  each 256-edge double-tile's src rows fit a static 32768-row table window
  (dma_gather int16 index limit).
- Per layer: node phase computes xl=[h@Wl | h@Wl@att*beta] and xr=h@Wr for the
  local shard; xl is AllGathered into a [50176, 256]-padded bf16 table.
- Edge phase per chunk: src rows fetched with one dma_gather per double-tile
  (the only descriptor-bound op; everything else rides other engines);
  xr[dst] is expanded from the chunk's xr rows with a one-hot matmul
  (S[j,e] = [dst_e == j]); e-term = ea (x) We via a K=1 matmul; all three sum
  in PSUM. LeakyReLU(0.2) is synthesized from the hardware's fixed-slope
  Lrelu(0.01): m = a*lrelu01(s) + b*s, with the linear part's head-reduction
  <s,att>*b carried as 8 extra gathered columns (dst-only terms cancel in the
  softmax, so xr's contribution is dropped). Softmax runs without max
  subtraction (exp of raw logits; mathematically identical, verified safe).
  Per-dst segment sums (num/den) are one-hot matmuls accumulated in PSUM.
- Pooled readout accumulates per layer in SBUF; host sums the 8 per-core
  partials and divides by per-graph counts.
"""
import numpy as np
import sys

sys.path.insert(0, '/opt/trn_rl_repo')

import ml_dtypes
from concourse import bass, mybir, tile, bacc
from concourse.bass_utils import run_bass_kernel_spmd

BF16 = ml_dtypes.bfloat16

N, E, F_IN, HID, H, C, L, G = 50000, 800000, 64, 128, 8, 16, 3, 64
NC_ = 8                    # cores
NPC = N // NC_             # 6250
NPAD = 6272                # 49 * 128
NCHUNK = NPAD // 128       # 49
TMAX = 18                  # edge tiles per chunk (validated for this data)
ND = TMAX // 2             # 9 double-tiles per chunk
EPC = NCHUNK * TMAX * 128  # padded edges per core
NROWS = NC_ * NPAD         # 50176 rows in the gathered src table
TBL_COLS = 256             # 512B rows: [xl 128 | ul 8 | pad]
WIN = 32768
NEG = 0.2
ALPHA = 0.8 / 0.99         # lrelu synthesis: m = ALPHA*lrelu01(s) + BETA*s
BETA = 1.0 - ALPHA

# static src-table window base per double-tile position (edges src-sorted
# within each chunk => double d covers roughly quantile [d/9,(d+1)/9] rows)
def _win_base(d):
    center = int(round((d + 0.5) / ND * NROWS))
    return max(0, min(NROWS - WIN, center - WIN // 2))

WBASE = [_win_base(d) for d in range(ND)]

_CACHE = {}


def _prep(x, edge_index, edge_attr, batch, w_pre1, b_pre1, w_pre2, b_pre2,
          Wl, Wr, We, att, bias_conv):
    """Host-side: shard + sort + pad edge data, prescale weights."""
    src = edge_index[0].astype(np.int64)
    dst = edge_index[1].astype(np.int64)
    ea = edge_attr.astype(np.float32)
    order = np.argsort(dst, kind='stable')
    src, dst, ea = src[order], dst[order], ea[order]
    row_of = (src // NPC) * NPAD + (src % NPC)   # padded-table row per edge

    att_f = att.reshape(L, HID).astype(np.float32)          # [L, 128]
    A_blk = np.zeros((L, HID, H), np.float32)               # blockdiag(att)*BETA
    for l in range(L):
        for h in range(H):
            A_blk[l, h * C:(h + 1) * C, h] = att[l, h] * BETA
    wa = np.einsum('lk,lkh->lh', We.astype(np.float32), A_blk)  # [L, 8]

    per_core = []
    for k in range(NC_):
        lo, hi = k * NPC, (k + 1) * NPC
        sel = (dst >= lo) & (dst < hi)
        s_k, d_k, e_k, r_k = src[sel], dst[sel] - lo, ea[sel], row_of[sel]
        chunk_of = d_k // 128
        counts = np.bincount(chunk_of, minlength=NCHUNK)
        assert counts.max() <= TMAX * 128, f"chunk overflow: {counts.max()}"

        idx16 = np.zeros((NCHUNK, TMAX * 128), np.int16)
        dstrow = np.full((NCHUNK, 2, TMAX * 128), -1.0, np.float32)
        dstrow[:, 1, :] = 1.0
        dstcol = np.full((NCHUNK * 128, TMAX), -1.0, np.float32)
        earow = np.zeros((NCHUNK, TMAX * 128), np.float32)
        base = 0
        for c in range(NCHUNK):
            n = counts[c]
            sl = slice(base, base + n)
            # sort this chunk's edges by src row for window locality
            so = np.argsort(r_k[sl], kind='stable')
            rows = r_k[sl][so]
            dl = (d_k[sl][so] - c * 128).astype(np.float32)
            ev = e_k[sl][so]
            # per-double window check + index rebase
            iv = np.zeros(TMAX * 128, np.int16)
            for dd in range(ND):
                a0, a1 = dd * 256, min((dd + 1) * 256, n)
                if a0 >= n:
                    break
                w = rows[a0:a1] - WBASE[dd]
                assert w.min() >= 0 and w.max() < WIN, \
                    f"window violation d={dd}: {w.min()}..{w.max()}"
                iv[a0:a1] = w.astype(np.int16)
            idx16[c] = iv
            dstrow[c, 0, :n] = dl
            dstcol[c * 128:(c + 1) * 128, :].T.flat[:n] = dl  # [e,t] = dl[t*128+e]
            earow[c, :n] = ev
            base += n
        # wrap idx into the dma_gather layout: j -> [j%16, j//16], per 256-idx
        # gather slice of 16 cols; replicate to 128 partitions
        idxw = np.zeros((16, NCHUNK * ND * 16), np.int16)
        for c in range(NCHUNK):
            for dd in range(ND):
                j = idx16[c, dd * 256:(dd + 1) * 256]
                idxw[:, (c * ND + dd) * 16:(c * ND + dd + 1) * 16] = \
                    j.reshape(16, 16).T
        idx_rep = np.tile(idxw, (8, 1))                       # [128, 7056]

        batch_col = np.full((128, NCHUNK), -1.0, np.float32)
        bk = batch[lo:hi].astype(np.float32)                  # [6250]
        batch_col.T.flat[:NPC] = bk                           # [e, c] = batch[c*128+e]

        xk = np.zeros((NPAD, F_IN), np.float32)
        xk[:NPC] = x[lo:hi]
        per_core.append(dict(
            idx=idx_rep,
            dstrow=dstrow.reshape(NCHUNK * 2, TMAX * 128).astype(BF16),
            dstcol=dstcol,
            earow=earow.astype(BF16).reshape(NCHUNK, TMAX * 128),
            batchcol=batch_col,
            xT=xk.T.copy().astype(BF16),                      # [64, 6272]
        ))

    # shared weights
    rep = lambda v: np.repeat(v.reshape(1, -1), 128, 0)
    shared = dict(
        w1=w_pre1.astype(BF16),                               # [64, 128]
        w2=w_pre2.astype(BF16),                               # [128, 128]
        bias1=rep(b_pre1).astype(np.float32),
        bias2=rep(b_pre2).astype(np.float32),
        iota_row=rep(np.arange(128, dtype=np.float32)).astype(BF16),
        iota64=rep(np.arange(64, dtype=np.float32))[:, :64].astype(BF16),
        c2=np.stack([np.ones(128, np.float32),
                     -np.arange(128, dtype=np.float32)]).astype(BF16),  # [2,128]
        ident=np.eye(128, dtype=np.float32).astype(BF16),
    )
    for l in range(L):
        Rl = np.concatenate([Wl[l], Wl[l] @ A_blk[l]], 1)     # [128, 136]
        shared[f'R{l}'] = Rl.astype(BF16)
        shared[f'Wr{l}'] = Wr[l].astype(BF16)
        shared[f'weext{l}'] = np.concatenate(
            [We[l], wa[l]]).reshape(1, HID + H).astype(BF16)  # [1, 136]
        shared[f'attrep{l}'] = rep(att_f[l]).astype(BF16)
        shared[f'biasc{l}'] = rep(bias_conv[l]).astype(np.float32)

    in_maps = [{**shared, **pc} for pc in per_core]
    return in_maps


def _build():
    nc = bacc.Bacc("TRN2", target_bir_lowering=False, debug=False,
                   num_devices=NC_)
    f32, bf16 = mybir.dt.float32, mybir.dt.bfloat16
    i16 = mybir.dt.int16
    AF = mybir.ActivationFunctionType
    OP = mybir.AluOpType
    ds = bass.ds

    # ---- inputs
    def inp(name, shape, dt):
        return nc.dram_tensor(name, shape, dt, kind="ExternalInput")

    xT = inp("xT", [F_IN, NPAD], bf16)
    w1 = inp("w1", [F_IN, HID], bf16)
    w2 = inp("w2", [HID, HID], bf16)
    bias1 = inp("bias1", [128, HID], f32)
    bias2 = inp("bias2", [128, HID], f32)
    iota_row = inp("iota_row", [128, 128], bf16)
    iota64 = inp("iota64", [128, 64], bf16)
    c2 = inp("c2", [2, 128], bf16)
    ident = inp("ident", [128, 128], bf16)
    R = [inp(f"R{l}", [HID, HID + H], bf16) for l in range(L)]
    Wr = [inp(f"Wr{l}", [HID, HID], bf16) for l in range(L)]
    weext = [inp(f"weext{l}", [1, HID + H], bf16) for l in range(L)]
    attrep = [inp(f"attrep{l}", [128, HID], bf16) for l in range(L)]
    biasc = [inp(f"biasc{l}", [128, HID], f32) for l in range(L)]
    idx_in = inp("idx", [128, NCHUNK * ND * 16], i16)
    dstrow_in = inp("dstrow", [NCHUNK * 2, TMAX * 128], bf16)
    dstcol_in = inp("dstcol", [NCHUNK * 128, TMAX], f32)
    earow_in = inp("earow", [NCHUNK, TMAX * 128], bf16)
    batchcol_in = inp("batchcol", [128, NCHUNK], f32)

    out_pools = nc.dram_tensor("out_pools", [64, L * HID], f32,
                               kind="ExternalOutput")

    # ---- internal DRAM
    hA = nc.dram_tensor("hA", [NPAD, HID], bf16)
    hB = nc.dram_tensor("hB", [NPAD, HID], bf16)
    xr_loc = nc.dram_tensor("xr_loc", [NPAD, HID], bf16)
    src_shard = nc.dram_tensor("src_shard", [NPAD, TBL_COLS], bf16)
    src_tbl = [nc.dram_tensor(f"src_tbl{l}", [NROWS, TBL_COLS], bf16,
                              addr_space="Shared") for l in range(L)]

    with tile.TileContext(nc) as tc:
        with tc.tile_pool(name="const", bufs=1) as cp, \
             tc.tile_pool(name="sbuf", bufs=3) as sb, \
             tc.tile_pool(name="psum", bufs=2, space="PSUM") as ps:

            # resident constants / metadata
            idx_sb = cp.tile([128, NCHUNK * ND * 16], i16)
            nc.sync.dma_start(out=idx_sb[:], in_=idx_in[:])
            batch_sb = cp.tile([128, NCHUNK], f32)
            nc.sync.dma_start(out=batch_sb[:], in_=batchcol_in[:])
            iota_sb = cp.tile([128, 128], bf16)
            nc.sync.dma_start(out=iota_sb[:], in_=iota_row[:])
            iota64_sb = cp.tile([128, 64], bf16)
            nc.sync.dma_start(out=iota64_sb[:], in_=iota64[:])
            c2_sb = cp.tile([2, 128], bf16)
            nc.sync.dma_start(out=c2_sb[:], in_=c2[:])
            id_sb = cp.tile([128, 128], bf16)
            nc.sync.dma_start(out=id_sb[:], in_=ident[:])
            w1_sb = cp.tile([F_IN, HID], bf16)
            nc.sync.dma_start(out=w1_sb[:], in_=w1[:])
            w2_sb = cp.tile([HID, HID], bf16)
            nc.sync.dma_start(out=w2_sb[:], in_=w2[:])
            b1_sb = cp.tile([128, HID], f32)
            nc.sync.dma_start(out=b1_sb[:], in_=bias1[:])
            b2_sb = cp.tile([128, HID], f32)
            nc.sync.dma_start(out=b2_sb[:], in_=bias2[:])
            R_sb, Wr_sb, we_sb, att_sb, bc_sb = [], [], [], [], []
            for l in range(L):
                t = cp.tile([HID, HID + H], bf16, name=f"Rsb{l}")
                nc.sync.dma_start(out=t[:], in_=R[l][:])
                R_sb.append(t)
                t = cp.tile([HID, HID], bf16, name=f"Wrsb{l}")
                nc.sync.dma_start(out=t[:], in_=Wr[l][:])
                Wr_sb.append(t)
                t = cp.tile([1, HID + H], bf16, name=f"wesb{l}")
                nc.sync.dma_start(out=t[:], in_=weext[l][:])
                we_sb.append(t)
                t = cp.tile([128, HID], bf16, name=f"attsb{l}")
                nc.sync.dma_start(out=t[:], in_=attrep[l][:])
                att_sb.append(t)
                t = cp.tile([128, HID], f32, name=f"bcsb{l}")
                nc.sync.dma_start(out=t[:], in_=biasc[l][:])
                bc_sb.append(t)
            pool_sb = cp.tile([64, L * HID], f32)
            nc.vector.memset(pool_sb[:], 0.0)

            # ---------- pre-MLP ----------
            def mlp_tile(i, lhsT_tile, rhs, bias_tile, hout):
                pm = ps.tile([128, HID], f32, tag="pss")
                nc.tensor.matmul(out=pm[:], lhsT=lhsT_tile, rhs=rhs,
                                 start=True, stop=True)
                o = sb.tile([128, HID], f32, tag="mlp_o")
                nc.vector.tensor_tensor(out=o[:], in0=pm[:], in1=bias_tile[:],
                                        op=OP.add)
                ob = sb.tile([128, HID], bf16, tag="mlp_ob")
                nc.scalar.activation(ob[:], o[:], AF.Relu)
                nc.sync.dma_start(out=hout[i * 128:(i + 1) * 128, :], in_=ob[:])

            for i in range(NCHUNK):
                xt = sb.tile([F_IN, 128], bf16, tag="xt")
                nc.sync.dma_start(out=xt[:], in_=xT[:, i * 128:(i + 1) * 128])
                mlp_tile(i, xt[:], w1_sb[:], b1_sb, hA)
            for i in range(NCHUNK):
                ht = sb.tile([128, 128], bf16, tag="ht")
                nc.sync.dma_start(out=ht[:], in_=hA[i * 128:(i + 1) * 128, :],
                                  transpose=True)
                mlp_tile(i, ht[:], w2_sb[:], b2_sb, hB)

            h_cur, h_nxt = hB, hA
            for l in range(L):
                # ---------- node phase ----------
                for i in range(NCHUNK):
                    ht = sb.tile([128, 128], bf16, tag="ht")
                    nc.sync.dma_start(
                        out=ht[:], in_=h_cur[i * 128:(i + 1) * 128, :],
                        transpose=True)
                    pa = ps.tile([128, HID + H], f32, tag="pss")
                    nc.tensor.matmul(out=pa[:], lhsT=ht[:], rhs=R_sb[l][:],
                                     start=True, stop=True)
                    pb = ps.tile([128, HID], f32, tag="pd2")
                    nc.tensor.matmul(out=pb[:], lhsT=ht[:], rhs=Wr_sb[l][:],
                                     start=True, stop=True)
                    xa = sb.tile([128, HID + H], bf16, tag="xa")
                    nc.scalar.activation(xa[:], pa[:], AF.Copy)
                    nc.sync.dma_start(
                        out=src_shard[i * 128:(i + 1) * 128, 0:HID + H],
                        in_=xa[:])
                    xb = sb.tile([128, HID], bf16, tag="xb")
                    nc.scalar.activation(xb[:], pb[:], AF.Copy)
                    nc.sync.dma_start(
                        out=xr_loc[i * 128:(i + 1) * 128, :], in_=xb[:])
                # ---------- allgather ----------
                nc.gpsimd.collective_compute(
                    "AllGather", OP.bypass,
                    replica_groups=[list(range(NC_))],
                    ins=[src_shard[:, :]], outs=[src_tbl[l][:, :]])

                # ---------- edge phase ----------
                def chunk_body(ci):
                    d2r = sb.tile([2, TMAX * 128], bf16, tag="d2r")
                    nc.sync.dma_start(out=d2r[:],
                                      in_=dstrow_in[ds(ci * 2, 2), :])
                    dcol = sb.tile([128, TMAX], f32, tag="dcol")
                    nc.sync.dma_start(out=dcol[:],
                                      in_=dstcol_in[ds(ci * 128, 128), :])
                    ear = sb.tile([1, TMAX * 128], bf16, tag="ear")
                    nc.sync.dma_start(out=ear[:], in_=earow_in[ds(ci, 1), :])
                    xrc = sb.tile([128, HID], bf16, tag="xrc")
                    nc.sync.dma_start(out=xrc[:],
                                      in_=xr_loc[ds(ci * 128, 128), :])
                    nump = ps.tile([128, HID + H], f32, tag="nump")
                    for d in range(ND):
                        gt = sb.tile([128, 2, TBL_COLS], bf16, tag="gt",
                                     bufs=4)
                        nc.gpsimd.dma_gather(
                            out_ap=gt[:],
                            in_ap=src_tbl[l][WBASE[d]:WBASE[d] + WIN, :],
                            idxs_ap=idx_sb[:, ds(ci * (ND * 16) + d * 16, 16)],
                            num_idxs=256, num_idxs_reg=256,
                            elem_size=TBL_COLS)
                        pd2 = ps.tile([128, 256], f32, tag="pd2")
                        nc.tensor.matmul(
                            out=pd2[:], lhsT=c2_sb[:],
                            rhs=d2r[:, d * 256:(d + 1) * 256],
                            start=True, stop=True)
                        S = sb.tile([128, 256], bf16, tag="S")
                        nc.vector.tensor_scalar(
                            out=S[:], in0=pd2[:], scalar1=0.0, scalar2=None,
                            op0=OP.is_equal)
                        pss = ps.tile([128, 2, HID + H], f32, tag="pss")
                        for t in range(2):
                            nc.tensor.matmul(
                                out=pss[:, t, :],
                                lhsT=ear[0:1, d * 256 + t * 128:
                                         d * 256 + (t + 1) * 128],
                                rhs=we_sb[l][:], start=True, stop=False,
                                skip_group_check=True)
                            nc.tensor.matmul(
                                out=pss[:, t, 0:HID],
                                lhsT=S[:, t * 128:(t + 1) * 128],
                                rhs=xrc[:], start=False, stop=False,
                                skip_group_check=True)
                            nc.tensor.matmul(
                                out=pss[:, t, :],
                                lhsT=id_sb[:],
                                rhs=gt[:, t, 0:HID + H], start=False,
                                stop=True, skip_group_check=True)
                        m1 = sb.tile([128, 2, HID], bf16, tag="m1")
                        nc.scalar.activation(m1[:], pss[:, :, 0:HID],
                                             AF.Lrelu, scale=ALPHA)
                        am = sb.tile([128, 2, HID], bf16, tag="am")
                        nc.vector.tensor_tensor(
                            out=am[:], in0=m1[:],
                            in1=att_sb[l][:].rearrange(
                                "p (x c) -> p x c", x=1).to_broadcast(
                                [128, 2, HID]),
                            op=OP.mult)
                        red = sb.tile([128, 2, H], f32, tag="red")
                        nc.vector.reduce_sum(
                            out=red[:],
                            in_=am[:].rearrange("p t (h c) -> p t h c", h=H),
                            axis=mybir.AxisListType.X)
                        a2 = sb.tile([128, 2, H], f32, tag="a2")
                        nc.vector.tensor_tensor(
                            out=a2[:], in0=red[:], in1=pss[:, :, HID:HID + H],
                            op=OP.add)
                        pexp = sb.tile([128, 2, H, C], bf16, tag="pexp")
                        nc.scalar.activation(
                            pexp[:],
                            a2[:].rearrange("p t (h x) -> p t h x", x=1)
                            .to_broadcast([128, 2, H, C]),
                            AF.Exp)
                        Wt = sb.tile([128, 2, HID + H], bf16, tag="Wt")
                        nc.vector.tensor_tensor(
                            out=Wt[:, :, 0:HID].rearrange(
                                "p t (h c) -> p t h c", h=H),
                            in0=pexp[:],
                            in1=gt[:, :, 0:HID].rearrange(
                                "p t (h c) -> p t h c", h=H),
                            op=OP.mult)
                        nc.vector.tensor_copy(
                            Wt[:, :, HID:HID + H].rearrange(
                                "p t (h x) -> p t h x", x=1),
                            pexp[:, :, :, 0:1])
                        ST = sb.tile([128, 2, 128], bf16, tag="ST")
                        for t in range(2):
                            nc.vector.tensor_scalar(
                                out=ST[:, t, :], in0=iota_sb[:],
                                scalar1=dcol[:, d * 2 + t:d * 2 + t + 1],
                                scalar2=None, op0=OP.is_equal)
                        for t in range(2):
                            nc.tensor.matmul(
                                out=nump[:], lhsT=ST[:, t, :],
                                rhs=Wt[:, t, :],
                                start=(d == 0 and t == 0),
                                stop=(d == ND - 1 and t == 1))
                    # ---- finalize chunk
                    rden = sb.tile([128, H], f32, tag="rden")
                    nc.vector.tensor_scalar(
                        out=rden[:], in0=nump[:, HID:HID + H], scalar1=1e-16,
                        scalar2=None, op0=OP.add)
                    rrec = sb.tile([128, H], f32, tag="rrec")
                    nc.vector.reciprocal(out=rrec[:], in_=rden[:])
                    o1 = sb.tile([128, HID], f32, tag="o1")
                    nc.vector.tensor_tensor(
                        out=o1[:].rearrange("p (h c) -> p h c", h=H),
                        in0=nump[:, 0:HID].rearrange("p (h c) -> p h c", h=H),
                        in1=rrec[:].rearrange("p (h x) -> p h x", x=1)
                        .to_broadcast([128, H, C]),
                        op=OP.mult)
                    o2 = sb.tile([128, HID], f32, tag="o2")
                    nc.vector.tensor_tensor(out=o2[:], in0=o1[:],
                                            in1=bc_sb[l][:], op=OP.add)
                    hnx = sb.tile([128, HID], bf16, tag="hnx")
                    nc.scalar.activation(hnx[:], o2[:], AF.Relu)
                    nc.sync.dma_start(out=h_nxt[ds(ci * 128, 128), :],
                                      in_=hnx[:])
                    Sb = sb.tile([128, 64], bf16, tag="Sb")
                    nc.vector.tensor_scalar(
                        out=Sb[:], in0=iota64_sb[:],
                        scalar1=batch_sb[:, ds(ci, 1)], scalar2=None,
                        op0=OP.is_equal)
                    pp = ps.tile([64, HID], f32, tag="pd2")
                    nc.tensor.matmul(out=pp[:], lhsT=Sb[:], rhs=hnx[:],
                                     start=True, stop=True)
                    nc.vector.tensor_tensor(
                        out=pool_sb[:, l * HID:(l + 1) * HID],
                        in0=pool_sb[:, l * HID:(l + 1) * HID],
                        in1=pp[:], op=OP.add)

                with tc.For_i(0, NCHUNK, 1) as ci:
                    chunk_body(ci)

                h_cur, h_nxt = h_nxt, h_cur

            nc.sync.dma_start(out=out_pools[:, :], in_=pool_sb[:])

    nc.compile()
    return nc


def kernel(**inputs):
    key = 'nc'
    if key not in _CACHE:
        _CACHE[key] = _build()
    nc = _CACHE[key]
    inputs = {k: np.asarray(v) for k, v in inputs.items()}
    in_maps = _prep(**inputs)
    res = run_bass_kernel_spmd(nc, in_maps, list(range(NC_)))
    pools = np.zeros((64, L * HID), np.float64)
    for r in res.results:
        pools += r['out_pools'].astype(np.float64)
    cnt = np.maximum(np.bincount(inputs['batch'].astype(np.int64),
                                 minlength=G), 1).astype(np.float64)
    out = (pools / cnt[:, None]).astype(np.float32)
    return out
